# revision 9
# baseline (speedup 1.0000x reference)
"""Trainium2 Bass kernel for nn_KernelMachine (random Fourier features).

out[n,m] = sum_f sqrt(2/F) * cos(x_n . a_f + b_f) * W[f*M+m]

Strategy (data-parallel over 8 NeuronCores, N sharded, a/b/W replicated):

Per core (N_loc=4096, D=16, F=4096, M=16):
  1. m1 (PE, bf16 split):  t = (x @ a.T + b + pi/2) / (2*pi)  in PSUM fp32.
     x and a are split hi/lo in bf16 (3-term product) so t is accurate to
     ~2^-18; b (+pi/2, /2pi) rides as two extra contraction rows against
     ones-rows. K=50, one pass, full speed.
  2. DVE magic-round:      k = (t + 1.5*2^23) - 1.5*2^23  (exact rint), bf16.
  3. corr (PE):            t -= I @ k  accumulated into the same PSUM bank,
     giving s = t - rint(t) in [-0.5, 0.5]  (exact Sterbenz subtraction).
  4. ACT:                  phi = Sin(2*pi*s)  == cos(x.a + b), fp32(r) SBUF.
     (ScalarE Sin is only valid on [-pi, pi]; the mod-1 range reduction above
     makes the argument exact up to t's error.)
  5. m2 (PE, f32r):        outT[m, n] += (W*sqrt(2/F))[f,m].T @ phi[f, n]
     accumulated over the 32 f-chunks. f32r moving at N=512 is full rate.
  6. epilogue: PE-transpose outT [16,512] -> [512,16] and DMA out.
"""

import math

import numpy as np
import ml_dtypes

import concourse.bass as bass
import concourse.tile as tile
from concourse import bacc, mybir
from concourse.bass_utils import run_bass_kernel_spmd

F32 = mybir.dt.float32
F32R = mybir.dt.float32r
BF16 = mybir.dt.bfloat16
FP16 = mybir.dt.float16

N, D, F, M = 32768, 16, 4096, 16
NCORES = 8
NLOC = N // NCORES            # 4096 rows per core
FC = F // 128                 # 32 f-chunks of 128
NJ = NLOC // 512              # 8 n-chunks of 512

MAGIC = float(np.float32(1.5 * 2 ** 23))
import os as _os
M2_FP16 = _os.environ.get("M2_FP16", "0") == "1"
PHI_DT = None  # assigned after dtype aliases
TWO_PI = float(2.0 * np.pi)

PHI_DT = FP16 if M2_FP16 else F32R
WSC_DT = FP16 if M2_FP16 else F32R
W_PRESCALE = 256.0 if M2_FP16 else 1.0

LDW_OPT = _os.environ.get("LDW_OPT", "0") == "1"
if LDW_OPT:
    import concourse.bass_utils as _bu
    _orig_run_command = _bu.run_command
    def _patched_run_command(argv, **kw):
        argv = ["--enable-ldw-opt=true" if a == "--enable-ldw-opt=false" else a
                for a in argv]
        return _orig_run_command(argv, **kw)
    _bu.run_command = _patched_run_command

_CACHE = {}


def build_nc():
    nc = bacc.Bacc(None, target_bir_lowering=False)

    x_in = nc.dram_tensor("x_in", [NLOC, D], F32, kind="ExternalInput")
    apack_in = nc.dram_tensor("apack_in", [50, F], BF16, kind="ExternalInput")
    wsc_in = nc.dram_tensor("wsc_in", [128, FC, M], WSC_DT, kind="ExternalInput")
    negi_in = nc.dram_tensor("negi_in", [128, 128], BF16, kind="ExternalInput")
    ident_in = nc.dram_tensor("ident_in", [128, 128], F32, kind="ExternalInput")
    ones_in = nc.dram_tensor("ones_in", [2, NLOC], BF16, kind="ExternalInput")
    out_t = nc.dram_tensor("out", [NLOC, M], F32, kind="ExternalOutput")

    with tile.TileContext(nc) as tc:
        with (
            tc.tile_pool(name="const", bufs=1) as const,
            tc.tile_pool(name="kp", bufs=4) as kp,
            tc.tile_pool(name="php", bufs=6) as php,
            tc.tile_pool(name="osb", bufs=2) as osb,
            tc.tile_pool(name="pst", bufs=6, space="PSUM") as pst,
            tc.tile_pool(name="pso", bufs=1, space="PSUM") as pso,
        ):
            # ---------------- constants ----------------
            apack = const.tile([50, F], BF16, tag="apack")
            nc.sync.dma_start(out=apack, in_=apack_in[:])
            wsc = const.tile([128, FC, M], WSC_DT, tag="wsc")
            nc.sync.dma_start(out=wsc, in_=wsc_in[:])
            negi = const.tile([128, 128], BF16, tag="negi")
            nc.sync.dma_start(out=negi, in_=negi_in[:])
            ident = const.tile([128, 128], F32, tag="ident")
            nc.sync.dma_start(out=ident, in_=ident_in[:])

            # ---------------- x prologue ----------------
            # xf[p, c, d] = x[128c + p, d]
            xf = const.tile([128, FC, D], F32, tag="xf")
            nc.sync.dma_start(
                out=xf, in_=x_in[:].rearrange("(c p) d -> p c d", p=128)
            )
            xh_b = const.tile([128, FC, D], BF16, tag="xhb")
            nc.vector.tensor_copy(out=xh_b, in_=xf)
            xh32 = const.tile([128, FC, D], F32, tag="xh32")
            nc.vector.tensor_copy(out=xh32, in_=xh_b)
            xr = const.tile([128, FC, D], F32, tag="xr")
            nc.vector.tensor_tensor(
                out=xr, in0=xf, in1=xh32, op=mybir.AluOpType.subtract
            )

            # xpack rows: [xh(0:16), xl(16:32), xh(32:48), ones(48:50)]
            # Engine writes must start at a 32-aligned partition, so xl goes
            # through a base-0 temp and is DMA'd into rows 16:32.
            xpack = const.tile([50, NLOC], BF16, tag="xpack")
            xlT = const.tile([16, NLOC], BF16, tag="xlT")
            for g in range(NLOC // 512):
                tph = pst.tile([16, 512], F32, tag="t")
                for q in range(4):
                    c = 4 * g + q
                    nc.tensor.transpose(
                        tph[:, 128 * q:128 * (q + 1)], xh32[:, c, :], ident
                    )
                nc.scalar.copy(out=xpack[0:16, 512 * g:512 * (g + 1)], in_=tph)
                tpl = pst.tile([16, 512], F32, tag="t")
                for q in range(4):
                    c = 4 * g + q
                    nc.tensor.transpose(
                        tpl[:, 128 * q:128 * (q + 1)], xr[:, c, :], ident
                    )
                nc.scalar.copy(out=xlT[:, 512 * g:512 * (g + 1)], in_=tpl)
            nc.sync.dma_start(out=xpack[16:32, :], in_=xlT)
            # duplicate xh rows into 32:48 (sbuf->sbuf DMA)
            nc.sync.dma_start(out=xpack[32:48, :], in_=xpack[0:16, :])
            nc.sync.dma_start(out=xpack[48:50, :], in_=ones_in[:])

            # ---------------- main loop (software-pipelined) ----------------
            # 512-column chunks ch = j*FC + c (j = n-chunk, c = f-chunk).
            # Stages, skewed so no engine waits on another within ~2 chunks:
            #   A: m1(ch)        PE   t[ch] = apack_c.T @ xpack_j   (psum)
            #   B: round(ch-1)   DVE  k = (t+M)-M  -> bf16 sbuf
            #   C: corr(ch-2)    PE   t -= I @ k   (same psum group)
            #   D: sin(ch-3)     ACT  phi = Sin(2*pi*t) -> fp16 sbuf
            #   E: m2(ch-5)      PE   out[j%2] += wsc_c.T @ phi
            NCH = NJ * FC                  # 256 chunks
            t_tiles = {}
            k_tiles = {}
            phi_tiles = {}

            outa = pso.tile([16, 512], F32, tag="oa")
            outb = pso.tile([16, 512], F32, tag="ob")
            outbanks = [outa, outb]

            def emit_epilogue(j):
                # outT[m, n'] (m<16) -> DVE 32x32 block transpose ->
                # blockT[i, 32*cb + m] = outT[m, 32*cb + i]; DMA scatters the
                # m<16 columns of each block to out[n, m].
                out_ps = outbanks[j % 2]
                outT = osb.tile([32, 512], F32, tag="outT")
                nc.gpsimd.memset(outT, 0.0)
                nc.scalar.mul(outT[0:16, :], out_ps, 1.0 / W_PRESCALE)
                blockT = osb.tile([32, 512], F32, tag="blockT")
                nc.vector.transpose(out=blockT, in_=outT)
                nc.sync.dma_start(
                    out=out_t[512 * j:512 * (j + 1), :].rearrange(
                        "(cb i) m -> i cb m", i=32
                    ),
                    in_=blockT.rearrange("p (cb jj) -> p cb jj", jj=32)[:, :, 0:M],
                )

            for it in range(NCH + 5):
                # A: m1
                if it < NCH:
                    ch = it
                    j, c = divmod(ch, FC)
                    tp = pst.tile([128, 512], F32, tag="t")
                    nc.tensor.matmul(
                        tp,
                        apack[:, 128 * c:128 * (c + 1)],
                        xpack[:, 512 * j:512 * (j + 1)],
                        start=True, stop=False,
                    )
                    t_tiles[ch] = tp
                # B: round
                if 1 <= it + 1 - 1 and 0 <= it - 1 < NCH:
                    ch = it - 1
                    k_bf = kp.tile([128, 512], BF16, tag="k")
                    nc.vector.tensor_scalar(
                        out=k_bf, in0=t_tiles[ch],
                        scalar1=MAGIC, scalar2=MAGIC,
                        op0=mybir.AluOpType.add, op1=mybir.AluOpType.subtract,
                    )
                    k_tiles[ch] = k_bf
                # C: corr
                if 0 <= it - 2 < NCH:
                    ch = it - 2
                    nc.tensor.matmul(
                        t_tiles[ch],
                        negi,
                        k_tiles.pop(ch),
                        start=False, stop=True,
                    )
                # D: sin
                if 0 <= it - 3 < NCH:
                    ch = it - 3
                    phi = php.tile([128, 512], PHI_DT, tag="phi")
                    nc.scalar.activation(
                        out=phi, in_=t_tiles.pop(ch),
                        func=mybir.ActivationFunctionType.Sin,
                        bias=0.0, scale=TWO_PI,
                    )
                    phi_tiles[ch] = phi
                # E: m2
                if 0 <= it - 5 < NCH:
                    ch = it - 5
                    j2, c2 = divmod(ch, FC)
                    nc.tensor.matmul(
                        outbanks[j2 % 2],
                        wsc[:, c2, :],
                        phi_tiles.pop(ch),
                        start=(c2 == 0), stop=(c2 == FC - 1),
                    )
                    if c2 == FC - 1:
                        emit_epilogue(j2)

    nc.finalize()
    return nc


def _host_prep(a, b, W):
    """Precompute replicated operand packs (float64 for exact splitting)."""
    inv2pi = 1.0 / (2.0 * np.pi)
    a64 = np.asarray(a, dtype=np.float64).T * inv2pi          # [16, F]
    ah = a64.astype(ml_dtypes.bfloat16)
    al = (a64 - ah.astype(np.float64)).astype(ml_dtypes.bfloat16)
    b64 = (np.asarray(b, dtype=np.float64) + np.pi / 2.0) * inv2pi  # [F]
    bh = b64.astype(ml_dtypes.bfloat16)
    bl = (b64 - bh.astype(np.float64)).astype(ml_dtypes.bfloat16)

    apack = np.zeros((50, F), dtype=ml_dtypes.bfloat16)
    apack[0:16] = ah       # pairs with xh
    apack[16:32] = ah      # pairs with xl
    apack[32:48] = al      # pairs with xh (dup rows)
    apack[48] = bh
    apack[49] = bl

    scale = math.sqrt(2.0 / F)
    wdt = np.float16 if M2_FP16 else np.float32
    W2 = (np.asarray(W, dtype=np.float64).reshape(F, M) * scale * W_PRESCALE).astype(wdt)
    wsc = np.ascontiguousarray(
        W2.reshape(FC, 128, M).transpose(1, 0, 2)
    )                                                          # [128, FC, M]

    negi = (-np.eye(128)).astype(ml_dtypes.bfloat16)
    ident = np.eye(128, dtype=np.float32)
    ones = np.ones((2, NLOC), dtype=ml_dtypes.bfloat16)
    return apack, wsc, negi, ident, ones


def kernel(x, a, b, W):
    x = np.ascontiguousarray(np.asarray(x, dtype=np.float32))
    apack, wsc, negi, ident, ones = _host_prep(a, b, W)

    if "nc" not in _CACHE:
        _CACHE["nc"] = build_nc()
    nc = _CACHE["nc"]

    in_maps = []
    for i in range(NCORES):
        in_maps.append({
            "x_in": np.ascontiguousarray(x[i * NLOC:(i + 1) * NLOC]),
            "apack_in": apack,
            "wsc_in": wsc,
            "negi_in": negi,
            "ident_in": ident,
            "ones_in": ones,
        })

    res = run_bass_kernel_spmd(nc, in_maps, core_ids=list(range(NCORES)))
    return np.concatenate([r["out"] for r in res.results], axis=0)
